# revision 2
# baseline (speedup 1.0000x reference)
"""Trainium2 Bass kernel for nn_NeuralTrustNetwork (gnn_message_passing).

out[e] = lrelu(lrelu(c) @ W_mlp + b_mlp) @ wL + bL
         + (x[src]*x[dst]) @ w1 + b1 + (w[src]*w[dst]) @ w2 + b2
  with c = (s1+s2)[src] + (p1+p2)[dst]

Strategy (edge-parallel across 8 NeuronCores, per the sharding hint):
- Host: build combined fp16 node tables SRC=[s1|s2|x|w], DST=[p1|p2|x|w]
  (512B rows), split into lo/hi halves (dma_gather indices are int16),
  bucket each core's edges by (src-half, dst-half), pad buckets to
  2048-edge batches.
- Device per 2048-edge batch: 4x dma_gather (1024 rows each — SWDGE
  descriptor-ring limit), DVE adds for c, PE pair-transposes + blockdiag
  W_mlp matmul for the MLP, fused LeakyReLU+bias on ACT, dot-product
  heads on DVE, everything accumulated into a [16,128] PSUM tile that
  stores contiguously.
"""

from contextlib import ExitStack

import numpy as np

import concourse.bacc as bacc
import concourse.bass as bass
import concourse.mybir as mybir
import concourse.tile as tile
from concourse.masks import make_identity

FP16 = mybir.dt.float16
FP8 = mybir.dt.float8e4
F32 = mybir.dt.float32
I16 = mybir.dt.int16

NCORES = 8
B = 2048          # edges per compute batch
BG = 1024         # edges per dma_gather (descriptor ring limit)
J = B // 128      # 16 slots
D = 64

_prog_cache = {}

# gather position i -> DRAM/out position q: q = (i%128)*16 + 8*(i//1024) + (i//128)%8
_I = np.arange(B)
_QPERM = (_I % 128) * (B // 128) + (B // 1024) * 4 * (_I // 1024) + (_I // 128) % 8
# inverse: gather list position i must hold stream edge (base + _QPERM[i])


def _wrap_idx_chunk(idx):
    """[1024] int -> [128, 64] int16 (wrap 16 partitions, replicate 8x)."""
    w = idx.reshape(-1, 16).T.astype(np.int16)  # [16, 64]
    return np.tile(w, (8, 1))


def _build_program(nb, nhalf, reps=1, variant='full'):
    """Build + compile the SPMD program for per-bucket batch counts nb[0..3]."""
    totb = sum(nb)
    nc = bacc.Bacc(
        "TRN2",
        target_bir_lowering=False,
        debug=False,
        enable_asserts=False,
        num_swdge_queues=4,
        dynamic_dma_scratch_size=131072,
    )
    src_lo = nc.dram_tensor("src_lo", [nhalf, 256], FP16, kind="ExternalInput").ap()
    src_hi = nc.dram_tensor("src_hi", [nhalf, 256], FP16, kind="ExternalInput").ap()
    dst_lo = nc.dram_tensor("dst_lo", [nhalf, 256], FP16, kind="ExternalInput").ap()
    dst_hi = nc.dram_tensor("dst_hi", [nhalf, 256], FP16, kind="ExternalInput").ap()
    idx_s = nc.dram_tensor("idx_s", [totb * 128, 128], I16, kind="ExternalInput").ap()
    idx_d = nc.dram_tensor("idx_d", [totb * 128, 128], I16, kind="ExternalInput").ap()
    wbd_d = nc.dram_tensor("wbd", [128, 128], FP16, kind="ExternalInput").ap()
    wlp_d = nc.dram_tensor("wlp", [128, 2], FP16, kind="ExternalInput").ap()
    bm2_d = nc.dram_tensor("bm2", [128, 1], F32, kind="ExternalInput").ap()
    b3_d = nc.dram_tensor("b3", [1, 3], F32, kind="ExternalInput").ap()
    out_d = nc.dram_tensor("out", [totb * 128, 16], F32, kind="ExternalOutput").ap()

    s_tabs = [src_lo, src_lo, src_hi, src_hi]
    d_tabs = [dst_lo, dst_hi, dst_lo, dst_hi]

    with tile.TileContext(nc) as tc, ExitStack() as ctx:
        const = ctx.enter_context(tc.tile_pool(name="const", bufs=1))
        ident = const.tile([128, 128], FP16)
        make_identity(nc, ident[:])
        wbd_t = const.tile([128, 128], FP16)
        nc.sync.dma_start(wbd_t[:], wbd_d[:])
        wlp_t = const.tile([128, 2], FP16)
        nc.sync.dma_start(wlp_t[:], wlp_d[:])
        bm2_t = const.tile([128, 1], F32)
        nc.sync.dma_start(bm2_t[:], bm2_d[:])
        b3_t = const.tile([1, 3], F32)
        nc.sync.dma_start(b3_t[:], b3_d[:])
        ones128f = const.tile([1, 128], F32)
        nc.vector.memset(ones128f[:], 1.0)

        k128 = const.tile([128, 1], F32)
        with tc.tile_pool(name="setup_ps", bufs=1, space="PSUM") as sps:
            pk = sps.tile([128, 3], F32)
            nc.tensor.matmul(pk[:], lhsT=ones128f[:], rhs=b3_t[:], start=True, stop=True)
            nc.vector.tensor_reduce(k128[:], pk[:], axis=mybir.AxisListType.X,
                                    op=mybir.AluOpType.add)

        idxp = ctx.enter_context(tc.tile_pool(name="idx", bufs=3))
        gp = ctx.enter_context(tc.tile_pool(name="gath", bufs=3))
        cp = ctx.enter_context(tc.tile_pool(name="csum", bufs=2))
        atp = ctx.enter_context(tc.tile_pool(name="at", bufs=4))
        l2p = ctx.enter_context(tc.tile_pool(name="l2", bufs=4))
        mp = ctx.enter_context(tc.tile_pool(name="m2", bufs=2))
        outp = ctx.enter_context(tc.tile_pool(name="outs", bufs=3))
        ps_c = ctx.enter_context(tc.tile_pool(name="ps_c", bufs=2, space="PSUM"))
        ps_h = ctx.enter_context(tc.tile_pool(name="ps_h", bufs=2, space="PSUM"))
        ps_o = ctx.enter_context(tc.tile_pool(name="ps_o", bufs=2, space="PSUM"))

        for rep in range(reps):
          t = 0
          for k in range(4):
            s_tab, d_tab = s_tabs[k], d_tabs[k]
            for _ in range(nb[k]):
                si = idxp.tile([128, 128], I16, tag="si")
                nc.sync.dma_start(si[:], idx_s[t * 128:(t + 1) * 128, :])
                di = idxp.tile([128, 128], I16, tag="di")
                nc.sync.dma_start(di[:], idx_d[t * 128:(t + 1) * 128, :])

                S = gp.tile([128, J, 256], FP16, tag="S")
                Dt = gp.tile([128, J, 256], FP16, tag="D")
                for h in range(2 if variant != 'compute' else 0):
                    nc.gpsimd.dma_gather(
                        out_ap=S[:, h * 8:(h + 1) * 8, :], in_ap=s_tab[:],
                        idxs_ap=si[:, h * 64:(h + 1) * 64],
                        num_idxs=BG, num_idxs_reg=BG, elem_size=256,
                        queue_num=2 * h,
                    )
                    nc.gpsimd.dma_gather(
                        out_ap=Dt[:, h * 8:(h + 1) * 8, :], in_ap=d_tab[:],
                        idxs_ap=di[:, h * 64:(h + 1) * 64],
                        num_idxs=BG, num_idxs_reg=BG, elem_size=256,
                        queue_num=2 * h + 1,
                    )

                if variant == 'gather':
                    t += 1
                    continue
                # c = s[src] + p[dst]
                c = cp.tile([128, J, D], FP16, tag="c")
                nc.vector.tensor_tensor(c[:], S[:, :, 0:64],
                                        Dt[:, :, 0:64],
                                        op=mybir.AluOpType.add)

                # heads: m2 = sum_f a*x + b*w  [128, J]
                tmp2 = mp.tile([128, J, 128], FP16, tag="tmp2")
                nc.vector.tensor_tensor(tmp2[:], S[:, :, 64:192],
                                        Dt[:, :, 64:192],
                                        op=mybir.AluOpType.mult)
                m2 = mp.tile([128, J], F32, tag="m2")
                nc.vector.tensor_reduce(m2[:], tmp2[:], axis=mybir.AxisListType.X,
                                        op=mybir.AluOpType.add)
                # MLP path per slot-pair; e1 accumulates edge-major [128, J]
                e1 = ps_o.tile([128, J], F32)
                for u in range(8):
                    pc = ps_c.tile([128, 128], FP16, tag="pc")
                    nc.tensor.matmul(pc[:], lhsT=c[:, 2 * u:2 * u + 2, :],
                                     rhs=ident[:], is_transpose=True,
                                     start=True, stop=True)
                    at = atp.tile([128, 128], FP16, tag="at")
                    nc.scalar.activation(at[:], pc[:],
                                         mybir.ActivationFunctionType.Lrelu,
                                         alpha=0.01)
                    ph = ps_h.tile([128, 128], F32, tag="ph")
                    nc.tensor.matmul(ph[:], lhsT=wbd_t[:], rhs=at[:],
                                     start=True, stop=True)
                    l2 = l2p.tile([128, 128], FP16, tag="l2")
                    nc.scalar.activation(l2[:], ph[:],
                                         mybir.ActivationFunctionType.Lrelu,
                                         bias=bm2_t[:, 0:1], alpha=0.01)
                    nc.tensor.matmul(e1[:, 2 * u:2 * u + 2], lhsT=l2[:],
                                     rhs=wlp_t[:], start=True, stop=True)

                ot = outp.tile([128, J], F32)
                nc.vector.scalar_tensor_tensor(
                    ot[:], e1[:], k128[:, 0:1], m2[:],
                    op0=mybir.AluOpType.add, op1=mybir.AluOpType.add)
                nc.sync.dma_start(out_d[t * 128:(t + 1) * 128, :], ot[:])
                t += 1

    nc.compile()
    return nc


def _prep(inputs):
    src = np.asarray(inputs["src"]).astype(np.int64).ravel()
    dst = np.asarray(inputs["dst"]).astype(np.int64).ravel()
    s1 = np.asarray(inputs["s1"], np.float32)
    s2 = np.asarray(inputs["s2"], np.float32)
    p1 = np.asarray(inputs["p1"], np.float32)
    p2 = np.asarray(inputs["p2"], np.float32)
    x = np.asarray(inputs["x"], np.float32)
    w = np.asarray(inputs["w"], np.float32)

    E = src.shape[0]
    N = s1.shape[0]
    assert E % NCORES == 0
    epc = E // NCORES
    nhalf = (N + 1) // 2

    w1 = np.asarray(inputs["w1"], np.float32).ravel()
    w2 = np.asarray(inputs["w2"], np.float32).ravel()
    zpad = np.zeros_like(x)
    src_tab = np.concatenate(
        [s1 + s2, x * w1[None, :], w * w2[None, :], zpad], axis=1
    ).astype(np.float16)
    dst_tab = np.concatenate([p1 + p2, x, w, zpad], axis=1).astype(np.float16)
    if N < 2 * nhalf:
        padrow = np.zeros((2 * nhalf - N, 256), np.float16)
        src_tab = np.vstack([src_tab, padrow])
        dst_tab = np.vstack([dst_tab, padrow])

    # bucket per core
    per_core = []
    counts = np.zeros((NCORES, 4), np.int64)
    for c in range(NCORES):
        s = src[c * epc:(c + 1) * epc]
        d = dst[c * epc:(c + 1) * epc]
        b = (s >= nhalf) * 2 + (d >= nhalf)
        ords = [np.flatnonzero(b == k) for k in range(4)]
        counts[c] = [len(o) for o in ords]
        per_core.append((s, d, ords))

    nb = [int(-(-counts[:, k].max() // B)) for k in range(4)]
    totb = sum(nb)

    idx_s_all = np.zeros((NCORES, totb * 128, 128), np.int16)
    idx_d_all = np.zeros((NCORES, totb * 128, 128), np.int16)
    order_all = np.full((NCORES, totb * B), -1, np.int64)

    for c in range(NCORES):
        s, d, ords = per_core[c]
        t = 0
        pos = 0
        for k in range(4):
            ids = ords[k]
            cap = nb[k] * B
            se = np.zeros(cap, np.int64)
            de = np.zeros(cap, np.int64)
            se[:len(ids)] = s[ids] - (nhalf if k >= 2 else 0)
            de[:len(ids)] = d[ids] - (nhalf if k % 2 == 1 else 0)
            order_all[c, pos:pos + len(ids)] = ids
            pos += cap
            for bi in range(nb[k]):
                blk_s = np.empty((128, 128), np.int16)
                blk_d = np.empty((128, 128), np.int16)
                seg_s = se[bi * B + _QPERM]
                seg_d = de[bi * B + _QPERM]
                for h in range(2):
                    sl = slice(h * BG, (h + 1) * BG)
                    blk_s[:, h * 64:(h + 1) * 64] = _wrap_idx_chunk(seg_s[sl])
                    blk_d[:, h * 64:(h + 1) * 64] = _wrap_idx_chunk(seg_d[sl])
                idx_s_all[c, t * 128:(t + 1) * 128] = blk_s
                idx_d_all[c, t * 128:(t + 1) * 128] = blk_d
                t += 1

    # weights
    W_mlp = np.asarray(inputs["W_mlp"], np.float32)
    b_mlp = np.asarray(inputs["b_mlp"], np.float32).ravel()
    wL = np.asarray(inputs["wL"], np.float32).ravel()
    w1 = np.asarray(inputs["w1"], np.float32).ravel()
    w2 = np.asarray(inputs["w2"], np.float32).ravel()
    bL = float(np.asarray(inputs["bL"]).ravel()[0])
    b1 = float(np.asarray(inputs["b1"]).ravel()[0])
    b2 = float(np.asarray(inputs["b2"]).ravel()[0])

    wbd = np.zeros((128, 128), np.float16)
    wbd[:64, :64] = W_mlp.astype(np.float16)
    wbd[64:, 64:] = W_mlp.astype(np.float16)
    wlp = np.zeros((128, 2), np.float16)
    wlp[:64, 0] = wL.astype(np.float16)
    wlp[64:, 1] = wL.astype(np.float16)
    bm2 = np.concatenate([b_mlp, b_mlp]).astype(np.float32).reshape(128, 1)
    b3 = np.array([[bL, b1, b2]], np.float32)

    weights = dict(wbd=wbd, wlp=wlp, bm2=bm2, b3=b3)
    tabs = dict(
        src_lo=np.ascontiguousarray(src_tab[:nhalf]),
        src_hi=np.ascontiguousarray(src_tab[nhalf:]),
        dst_lo=np.ascontiguousarray(dst_tab[:nhalf]),
        dst_hi=np.ascontiguousarray(dst_tab[nhalf:]),
    )
    return (tuple(nb), nhalf, epc, E, tabs, weights,
            idx_s_all, idx_d_all, order_all)


def run(inputs, **spmd_kwargs):
    """Returns (output [E,1] float32, BassKernelResults)."""
    from concourse.bass_utils import run_bass_kernel_spmd

    (nb, nhalf, epc, E, tabs, weights,
     idx_s_all, idx_d_all, order_all) = _prep(inputs)

    key = (nb, nhalf)
    if key not in _prog_cache:
        _prog_cache[key] = _build_program(list(nb), nhalf)
    nc = _prog_cache[key]

    in_maps = []
    for c in range(NCORES):
        m = dict(tabs)
        m.update(weights)
        m["idx_s"] = idx_s_all[c]
        m["idx_d"] = idx_d_all[c]
        in_maps.append(m)

    res = run_bass_kernel_spmd(nc, in_maps, list(range(NCORES)), **spmd_kwargs)

    out = np.empty((E, 1), np.float32)
    for c in range(NCORES):
        oc = np.asarray(res.results[c]["out"], np.float32).reshape(-1)
        order = order_all[c]
        valid = order >= 0
        out[c * epc + order[valid], 0] = oc[valid]
    return out, res


def kernel(**inputs) -> np.ndarray:
    out, _ = run(inputs)
    return out



# revision 4
# speedup vs baseline: 520.4901x; 520.4901x over previous
"""Trainium2 Bass kernel for nn_NeuralTrustNetwork (gnn_message_passing).

out[e] = lrelu(lrelu(c) @ W_mlp + b_mlp) @ wL + bL
         + (x[src]*x[dst]) @ w1 + b1 + (w[src]*w[dst]) @ w2 + b2
  with c = (s1+s2)[src] + (p1+p2)[dst]

Strategy (edge-parallel across 8 NeuronCores, per the sharding hint):
- Host: build combined fp16 node tables SRC=[s1+s2 | x*w1 | w*w2 | 0],
  DST=[p1+p2 | x | w | 0] (512B rows; node-level sums and head weights
  folded on host), split into lo/hi halves (dma_gather indices are
  int16), bucket each core's edges by (src-half, dst-half), pad buckets
  to 2048-edge batches.
- Device per 2048-edge batch: 4x dma_gather (1024 rows each — SWDGE
  descriptor-ring limit), one DVE add for c, one DVE mult + free-dim
  reduce for both dot-product heads, PE pair-transposes + blockdiag
  W_mlp matmul for the MLP, fused LeakyReLU+bias on ACT, heads and
  biases fused into the output via scalar_tensor_tensor, everything
  accumulated into a [16,128] PSUM tile that stores contiguously.
"""

from contextlib import ExitStack

import numpy as np

import concourse.bacc as bacc
import concourse.bass as bass
import concourse.mybir as mybir
import concourse.tile as tile
from concourse.masks import make_identity

FP16 = mybir.dt.float16
FP8 = mybir.dt.float8e4
F32 = mybir.dt.float32
I16 = mybir.dt.int16

NCORES = 8
B = 2048          # edges per compute batch
BG = 1024         # edges per dma_gather (descriptor ring limit)
J = B // 128      # 16 slots
D = 64

_prog_cache = {}

# gather position i -> DRAM/out position q: q = (i%128)*16 + 8*(i//1024) + (i//128)%8
_I = np.arange(B)
_QPERM = (_I % 128) * (B // 128) + (B // 1024) * 4 * (_I // 1024) + (_I // 128) % 8
# inverse: gather list position i must hold stream edge (base + _QPERM[i])


def _wrap_idx_chunk(idx):
    """[1024] int -> [128, 64] int16 (wrap 16 partitions, replicate 8x)."""
    w = idx.reshape(-1, 16).T.astype(np.int16)  # [16, 64]
    return np.tile(w, (8, 1))


def _build_program(nb, nhalf, reps=1, variant='full'):
    """Build + compile the SPMD program for per-bucket batch counts nb[0..3]."""
    totb = sum(nb)
    nc = bacc.Bacc(
        "TRN2",
        target_bir_lowering=False,
        debug=False,
        enable_asserts=False,
        num_swdge_queues=4,
    )
    src_lo = nc.dram_tensor("src_lo", [nhalf, 256], FP16, kind="ExternalInput").ap()
    src_hi = nc.dram_tensor("src_hi", [nhalf, 256], FP16, kind="ExternalInput").ap()
    dst_lo = nc.dram_tensor("dst_lo", [nhalf, 256], FP16, kind="ExternalInput").ap()
    dst_hi = nc.dram_tensor("dst_hi", [nhalf, 256], FP16, kind="ExternalInput").ap()
    idx_s = nc.dram_tensor("idx_s", [totb * 128, 128], I16, kind="ExternalInput").ap()
    idx_d = nc.dram_tensor("idx_d", [totb * 128, 128], I16, kind="ExternalInput").ap()
    wbd_d = nc.dram_tensor("wbd", [128, 128], FP16, kind="ExternalInput").ap()
    wlp_d = nc.dram_tensor("wlp", [128, 2], FP16, kind="ExternalInput").ap()
    bm2_d = nc.dram_tensor("bm2", [128, 1], F32, kind="ExternalInput").ap()
    b3_d = nc.dram_tensor("b3", [1, 3], F32, kind="ExternalInput").ap()
    out_d = nc.dram_tensor("out", [totb * 128, 16], F32, kind="ExternalOutput").ap()

    s_tabs = [src_lo, src_lo, src_hi, src_hi]
    d_tabs = [dst_lo, dst_hi, dst_lo, dst_hi]

    with tile.TileContext(nc) as tc, ExitStack() as ctx:
        const = ctx.enter_context(tc.tile_pool(name="const", bufs=1))
        ident = const.tile([128, 128], FP16)
        make_identity(nc, ident[:])
        wbd_t = const.tile([128, 128], FP16)
        nc.sync.dma_start(wbd_t[:], wbd_d[:])
        wlp_t = const.tile([128, 2], FP16)
        nc.sync.dma_start(wlp_t[:], wlp_d[:])
        bm2_t = const.tile([128, 1], F32)
        nc.sync.dma_start(bm2_t[:], bm2_d[:])
        b3_t = const.tile([1, 3], F32)
        nc.sync.dma_start(b3_t[:], b3_d[:])
        ones128f = const.tile([1, 128], F32)
        nc.vector.memset(ones128f[:], 1.0)

        k128 = const.tile([128, 1], F32)
        with tc.tile_pool(name="setup_ps", bufs=1, space="PSUM") as sps:
            pk = sps.tile([128, 3], F32)
            nc.tensor.matmul(pk[:], lhsT=ones128f[:], rhs=b3_t[:], start=True, stop=True)
            nc.vector.tensor_reduce(k128[:], pk[:], axis=mybir.AxisListType.X,
                                    op=mybir.AluOpType.add)

        idxp = ctx.enter_context(tc.tile_pool(name="idx", bufs=3))
        gp = ctx.enter_context(tc.tile_pool(name="gath", bufs=3))
        cp = ctx.enter_context(tc.tile_pool(name="csum", bufs=2))
        atp = ctx.enter_context(tc.tile_pool(name="at", bufs=4))
        l2p = ctx.enter_context(tc.tile_pool(name="l2", bufs=4))
        mp = ctx.enter_context(tc.tile_pool(name="m2", bufs=2))
        outp = ctx.enter_context(tc.tile_pool(name="outs", bufs=3))
        ps_c = ctx.enter_context(tc.tile_pool(name="ps_c", bufs=2, space="PSUM"))
        ps_h = ctx.enter_context(tc.tile_pool(name="ps_h", bufs=2, space="PSUM"))
        ps_o = ctx.enter_context(tc.tile_pool(name="ps_o", bufs=2, space="PSUM"))

        for rep in range(reps):
          t = 0
          for k in range(4):
            s_tab, d_tab = s_tabs[k], d_tabs[k]
            for _ in range(nb[k]):
                si = idxp.tile([128, 128], I16, tag="si")
                nc.sync.dma_start(si[:], idx_s[t * 128:(t + 1) * 128, :])
                di = idxp.tile([128, 128], I16, tag="di")
                nc.sync.dma_start(di[:], idx_d[t * 128:(t + 1) * 128, :])

                S = gp.tile([128, J, 256], FP16, tag="S")
                Dt = gp.tile([128, J, 256], FP16, tag="D")
                for h in range(2 if variant != 'compute' else 0):
                    nc.gpsimd.dma_gather(
                        out_ap=S[:, h * 8:(h + 1) * 8, :], in_ap=s_tab[:],
                        idxs_ap=si[:, h * 64:(h + 1) * 64],
                        num_idxs=BG, num_idxs_reg=BG, elem_size=256,
                        queue_num=2 * h,
                    )
                    nc.gpsimd.dma_gather(
                        out_ap=Dt[:, h * 8:(h + 1) * 8, :], in_ap=d_tab[:],
                        idxs_ap=di[:, h * 64:(h + 1) * 64],
                        num_idxs=BG, num_idxs_reg=BG, elem_size=256,
                        queue_num=2 * h + 1,
                    )

                if variant == 'gather':
                    t += 1
                    continue
                # c = s[src] + p[dst]
                c = cp.tile([128, J, D], FP16, tag="c")
                nc.vector.tensor_tensor(c[:], S[:, :, 0:64],
                                        Dt[:, :, 0:64],
                                        op=mybir.AluOpType.add)

                # heads: m2 = sum_f a*x + b*w  [128, J]
                tmp2 = mp.tile([128, J, 128], FP16, tag="tmp2")
                nc.vector.tensor_tensor(tmp2[:], S[:, :, 64:192],
                                        Dt[:, :, 64:192],
                                        op=mybir.AluOpType.mult)
                m2 = mp.tile([128, J], F32, tag="m2")
                nc.vector.tensor_reduce(m2[:], tmp2[:], axis=mybir.AxisListType.X,
                                        op=mybir.AluOpType.add)
                # MLP path per slot-pair; e1 accumulates edge-major [128, J]
                e1 = ps_o.tile([128, J], F32)
                for u in range(8):
                    pc = ps_c.tile([128, 128], FP16, tag="pc")
                    nc.tensor.matmul(pc[:], lhsT=c[:, 2 * u:2 * u + 2, :],
                                     rhs=ident[:], is_transpose=True,
                                     start=True, stop=True)
                    at = atp.tile([128, 128], FP16, tag="at")
                    nc.scalar.activation(at[:], pc[:],
                                         mybir.ActivationFunctionType.Lrelu,
                                         alpha=0.01)
                    ph = ps_h.tile([128, 128], F32, tag="ph")
                    nc.tensor.matmul(ph[:], lhsT=wbd_t[:], rhs=at[:],
                                     start=True, stop=True)
                    l2 = l2p.tile([128, 128], FP16, tag="l2")
                    nc.scalar.activation(l2[:], ph[:],
                                         mybir.ActivationFunctionType.Lrelu,
                                         bias=bm2_t[:, 0:1], alpha=0.01)
                    nc.tensor.matmul(e1[:, 2 * u:2 * u + 2], lhsT=l2[:],
                                     rhs=wlp_t[:], start=True, stop=True)

                ot = outp.tile([128, J], F32)
                nc.vector.scalar_tensor_tensor(
                    ot[:], e1[:], k128[:, 0:1], m2[:],
                    op0=mybir.AluOpType.add, op1=mybir.AluOpType.add)
                nc.sync.dma_start(out_d[t * 128:(t + 1) * 128, :], ot[:])
                t += 1

    nc.compile()
    return nc


def _prep(inputs):
    src = np.asarray(inputs["src"]).astype(np.int64).ravel()
    dst = np.asarray(inputs["dst"]).astype(np.int64).ravel()
    s1 = np.asarray(inputs["s1"], np.float32)
    s2 = np.asarray(inputs["s2"], np.float32)
    p1 = np.asarray(inputs["p1"], np.float32)
    p2 = np.asarray(inputs["p2"], np.float32)
    x = np.asarray(inputs["x"], np.float32)
    w = np.asarray(inputs["w"], np.float32)

    E = src.shape[0]
    N = s1.shape[0]
    assert E % NCORES == 0
    epc = E // NCORES
    nhalf = (N + 1) // 2

    w1 = np.asarray(inputs["w1"], np.float32).ravel()
    w2 = np.asarray(inputs["w2"], np.float32).ravel()
    zpad = np.zeros_like(x)
    src_tab = np.concatenate(
        [s1 + s2, x * w1[None, :], w * w2[None, :], zpad], axis=1
    ).astype(np.float16)
    dst_tab = np.concatenate([p1 + p2, x, w, zpad], axis=1).astype(np.float16)
    if N < 2 * nhalf:
        padrow = np.zeros((2 * nhalf - N, 256), np.float16)
        src_tab = np.vstack([src_tab, padrow])
        dst_tab = np.vstack([dst_tab, padrow])

    # bucket per core
    per_core = []
    counts = np.zeros((NCORES, 4), np.int64)
    for c in range(NCORES):
        s = src[c * epc:(c + 1) * epc]
        d = dst[c * epc:(c + 1) * epc]
        b = (s >= nhalf) * 2 + (d >= nhalf)
        ords = [np.flatnonzero(b == k) for k in range(4)]
        counts[c] = [len(o) for o in ords]
        per_core.append((s, d, ords))

    nb = [int(-(-counts[:, k].max() // B)) for k in range(4)]
    totb = sum(nb)

    idx_s_all = np.zeros((NCORES, totb * 128, 128), np.int16)
    idx_d_all = np.zeros((NCORES, totb * 128, 128), np.int16)
    order_all = np.full((NCORES, totb * B), -1, np.int64)

    for c in range(NCORES):
        s, d, ords = per_core[c]
        t = 0
        pos = 0
        for k in range(4):
            ids = ords[k]
            cap = nb[k] * B
            se = np.zeros(cap, np.int64)
            de = np.zeros(cap, np.int64)
            se[:len(ids)] = s[ids] - (nhalf if k >= 2 else 0)
            de[:len(ids)] = d[ids] - (nhalf if k % 2 == 1 else 0)
            order_all[c, pos:pos + len(ids)] = ids
            pos += cap
            for bi in range(nb[k]):
                blk_s = np.empty((128, 128), np.int16)
                blk_d = np.empty((128, 128), np.int16)
                seg_s = se[bi * B + _QPERM]
                seg_d = de[bi * B + _QPERM]
                for h in range(2):
                    sl = slice(h * BG, (h + 1) * BG)
                    blk_s[:, h * 64:(h + 1) * 64] = _wrap_idx_chunk(seg_s[sl])
                    blk_d[:, h * 64:(h + 1) * 64] = _wrap_idx_chunk(seg_d[sl])
                idx_s_all[c, t * 128:(t + 1) * 128] = blk_s
                idx_d_all[c, t * 128:(t + 1) * 128] = blk_d
                t += 1

    # weights
    W_mlp = np.asarray(inputs["W_mlp"], np.float32)
    b_mlp = np.asarray(inputs["b_mlp"], np.float32).ravel()
    wL = np.asarray(inputs["wL"], np.float32).ravel()
    w1 = np.asarray(inputs["w1"], np.float32).ravel()
    w2 = np.asarray(inputs["w2"], np.float32).ravel()
    bL = float(np.asarray(inputs["bL"]).ravel()[0])
    b1 = float(np.asarray(inputs["b1"]).ravel()[0])
    b2 = float(np.asarray(inputs["b2"]).ravel()[0])

    wbd = np.zeros((128, 128), np.float16)
    wbd[:64, :64] = W_mlp.astype(np.float16)
    wbd[64:, 64:] = W_mlp.astype(np.float16)
    wlp = np.zeros((128, 2), np.float16)
    wlp[:64, 0] = wL.astype(np.float16)
    wlp[64:, 1] = wL.astype(np.float16)
    bm2 = np.concatenate([b_mlp, b_mlp]).astype(np.float32).reshape(128, 1)
    b3 = np.array([[bL, b1, b2]], np.float32)

    weights = dict(wbd=wbd, wlp=wlp, bm2=bm2, b3=b3)
    tabs = dict(
        src_lo=np.ascontiguousarray(src_tab[:nhalf]),
        src_hi=np.ascontiguousarray(src_tab[nhalf:]),
        dst_lo=np.ascontiguousarray(dst_tab[:nhalf]),
        dst_hi=np.ascontiguousarray(dst_tab[nhalf:]),
    )
    return (tuple(nb), nhalf, epc, E, tabs, weights,
            idx_s_all, idx_d_all, order_all)


def run(inputs, **spmd_kwargs):
    """Returns (output [E,1] float32, BassKernelResults)."""
    from concourse.bass_utils import run_bass_kernel_spmd

    (nb, nhalf, epc, E, tabs, weights,
     idx_s_all, idx_d_all, order_all) = _prep(inputs)

    key = (nb, nhalf)
    if key not in _prog_cache:
        _prog_cache[key] = _build_program(list(nb), nhalf)
    nc = _prog_cache[key]

    in_maps = []
    for c in range(NCORES):
        m = dict(tabs)
        m.update(weights)
        m["idx_s"] = idx_s_all[c]
        m["idx_d"] = idx_d_all[c]
        in_maps.append(m)

    res = run_bass_kernel_spmd(nc, in_maps, list(range(NCORES)), **spmd_kwargs)

    out = np.empty((E, 1), np.float32)
    for c in range(NCORES):
        oc = np.asarray(res.results[c]["out"], np.float32).reshape(-1)
        order = order_all[c]
        valid = order >= 0
        out[c * epc + order[valid], 0] = oc[valid]
    return out, res


def kernel(**inputs) -> np.ndarray:
    out, _ = run(inputs)
    return out

